# revision 16
# baseline (speedup 1.0000x reference)
"""Trainium2 Bass kernel for CrossAttention (self-attention) nn module.

Reference computation (B=2, N=4096, D=512, H=8, DH=64):
    q, k, v = x@Wq, x@Wk, x@Wv          # [B, N, 512]
    per head: S = q k^T / sqrt(64); P = softmax(S); O = P v
    out = concat_heads(O) @ Wo + bo     # [B, N, 512]

Sharding: batch*head-pair across 8 cores. Core c handles batch c//4 and
head pair c%4 (heads 2p, 2p+1). Each core computes its two heads'
attention plus its partial output projection O_pair @ Wo[rows]; the host
sums the four partials per batch and adds the bias.

Device-side layout strategy (per core):
  - Host supplies x[b]^T so the contraction dim (D) lands on partitions.
  - QT/KT computed head-transposed [128(2 heads x 64), 4096] via fp32r
    matmuls (full PE rate at N>=256).
  - S^T[keys, q] = K @ Q^T per head, K=64 row-packed: head0 in PE rows
    0-63, head1 in rows 64-127, running concurrently.
  - exp on ScalarE directly from PSUM ([128, 1024] tiles covering both
    heads) -- no max subtraction (scores are O(1); exp is shift-invariant
    under softmax and fp32 cannot overflow here).
  - PV uses V in natural layout augmented with a ones column (M=65): PSUM
    row 64 accumulates the softmax denominator for free.
  - normalize: reciprocal on DVE, broadcast via K=1 matmul, multiply on
    DVE; head1's O^T block is shifted to partitions 64-127 with a
    SBUF->SBUF DMA; output projection is a single K=128 matmul per
    128-token block.
"""

import os
import sys

import numpy as np

for _p in ("/opt/trn_rl_repo", "/root/.axon_site/_ro/trn_rl_repo"):
    if os.path.isdir(_p) and _p not in sys.path:
        sys.path.insert(0, _p)

import concourse.bass as bass  # noqa: E402
import concourse.mybir as mybir  # noqa: E402
from concourse import bacc  # noqa: E402
from concourse.bass_utils import run_bass_kernel_spmd  # noqa: E402
from concourse.tile import TileContext  # noqa: E402

B, N, D = 2, 4096, 512
H, DH = 8, 64
P = 128                 # SBUF partitions / token block
KB = N // P             # 32 key blocks
QC = N // 512           # 8 query column blocks of 512
KCH = D // P            # 4 contraction chunks for the projections
SCALE = DH ** -0.5
NCORES = 8
K_PRE = 4               # S^T/exp steps emitted before prev qc's epilogue

# knobs for test.py
TRACE = False
LAST_RESULT = None

_CACHED_NC = None


def build_nc():
    f32 = mybir.dt.float32
    f32r = mybir.dt.float32r
    Exp = mybir.ActivationFunctionType.Exp

    nc = bacc.Bacc()
    xT = nc.declare_dram_parameter("xT", [D, N], f32r, isOutput=False)
    wq = nc.declare_dram_parameter("wq", [D, P], f32r, isOutput=False)
    wk = nc.declare_dram_parameter("wk", [D, P], f32r, isOutput=False)
    wv = nc.declare_dram_parameter("wv", [D, P], f32r, isOutput=False)
    wo = nc.declare_dram_parameter("wo", [P, D], f32r, isOutput=False)
    ident_d = nc.declare_dram_parameter("ident", [P, P], f32r, isOutput=False)
    ones_d = nc.declare_dram_parameter("ones", [P, DH], f32r, isOutput=False)
    y = nc.declare_dram_parameter("y", [N, D], f32, isOutput=True)

    with TileContext(nc) as tc:
        with (
            tc.tile_pool(name="persist", bufs=1) as persist,
            tc.tile_pool(name="proj", bufs=1) as proj,
            tc.tile_pool(name="ptp", bufs=3) as ptp,
            tc.tile_pool(name="work", bufs=3) as work,
            tc.tile_pool(name="ps_big", bufs=2, space="PSUM") as ps_big,
            tc.tile_pool(name="ps_acc", bufs=2, space="PSUM") as ps_acc,
            tc.tile_pool(name="ps_small", bufs=2, space="PSUM") as ps_small,
        ):
            # ---------------- prologue: loads ----------------
            xt_sb = persist.tile([P, KCH, N], f32r, tag="xt")
            for c in range(KCH):
                nc.sync.dma_start(out=xt_sb[:, c, :], in_=xT[c * P:(c + 1) * P, :])

            wq_sb = persist.tile([P, KCH, P], f32r, tag="wq")
            wk_sb = persist.tile([P, KCH, P], f32r, tag="wk")
            wv_sb = persist.tile([P, KCH, P], f32r, tag="wv")
            for w_sb, w_d in ((wq_sb, wq), (wk_sb, wk), (wv_sb, wv)):
                nc.sync.dma_start(
                    out=w_sb, in_=w_d.rearrange("(c p) m -> p c m", p=P)
                )
            wo_sb = persist.tile([P, D], f32r, tag="wo")
            nc.sync.dma_start(out=wo_sb, in_=wo[:, :])

            ident = persist.tile([P, P], f32r, tag="ident")
            nc.sync.dma_start(out=ident, in_=ident_d[:, :])
            ones_t = persist.tile([P, DH], f32r, tag="ones")
            nc.sync.dma_start(out=ones_t, in_=ones_d[:, :])

            # ---------------- projections ----------------
            # QT/KT/VT: [128 (2 heads x 64 dims), 4096 tokens]
            qt = persist.tile([P, N], f32r, tag="qt")
            kt = persist.tile([P, N], f32r, tag="kt")
            vt = proj.tile([P, N], f32r, tag="vt")
            for dst, w_sb in ((qt, wq_sb), (kt, wk_sb), (vt, wv_sb)):
                for col in range(QC):
                    csl = slice(col * 512, (col + 1) * 512)
                    psp = ps_small.tile([P, 512], f32, tag="small")
                    for c in range(KCH):
                        nc.tensor.matmul(
                            psp,
                            lhsT=w_sb[:, c, :],
                            rhs=xt_sb[:, c, csl],
                            start=(c == 0),
                            stop=(c == KCH - 1),
                        )
                    nc.vector.tensor_copy(dst[:, csl], psp)

            # V natural layout + ones column: v_aug[:, kb, h, 0:64] = V block,
            # v_aug[:, kb, h, 64] = 1.0
            v_aug = persist.tile([P, KB, 2, DH + 1], f32r, tag="vaug")
            nc.sync.dma_start(
                out=v_aug[:, :, :, DH:DH + 1],
                in_=ones_d[:, :].rearrange("p (a b c) -> p a b c", a=KB, b=2),
            )
            for kb in range(KB):
                pst = ps_small.tile([P, P], f32r, tag="small")
                nc.tensor.transpose(pst, vt[:, kb * P:(kb + 1) * P], ident)
                nc.vector.tensor_copy(
                    v_aug[:, kb, :, 0:DH],
                    pst.rearrange("p (h d) -> p h d", h=2),
                )

            # ---------------- attention + output projection ----------------
            state = {}

            def emit_step(qc, kb):
                """S^T for both heads (row-packed) + exp."""
                qsl = slice(qc * 512, (qc + 1) * 512)
                ksl = slice(kb * P, (kb + 1) * P)
                ps_s = ps_big.tile([P, 1024], f32, tag="psS")
                nc.tensor.matmul(
                    ps_s[:, 0:512],
                    lhsT=kt[0:DH, ksl],
                    rhs=qt[0:DH, qsl],
                )
                nc.tensor.matmul(
                    ps_s[:, 512:1024],
                    lhsT=kt[DH:P, ksl],
                    rhs=qt[DH:P, qsl],
                )
                pt = ptp.tile([P, 1024], f32r, tag="pt")
                nc.scalar.activation(pt, ps_s, func=Exp, scale=SCALE)
                state[(qc, kb)] = pt

            def emit_pv(qc, kb):
                pt = state.pop((qc, kb))
                if kb == 0:
                    state[(qc, "o0")] = ps_acc.tile(
                        [DH + 1, 512], f32, tag="psO", name="ps_o0"
                    )
                    state[(qc, "o1")] = ps_acc.tile(
                        [DH + 1, 512], f32, tag="psO", name="ps_o1"
                    )
                for h in range(2):
                    nc.tensor.matmul(
                        state[(qc, "o0" if h == 0 else "o1")],
                        lhsT=v_aug[:, kb, h, :],
                        rhs=pt[:, h * 512:(h + 1) * 512],
                        start=(kb == 0),
                        stop=(kb == KB - 1),
                    )

            def emit_epilogue(qc):
                ps_o0 = state.pop((qc, "o0"))
                ps_o1 = state.pop((qc, "o1"))
                # reciprocal of the softmax sums (PSUM row 64 -> SBUF row 64)
                inv0 = work.tile([DH + 1, 512], f32r, tag="inv")
                inv1 = work.tile([DH + 1, 512], f32r, tag="inv")
                with nc.allow_low_precision(
                    reason="float32r is fp32 storage; matmul operands need f32r"
                ):
                    nc.vector.reciprocal(inv0[DH:DH + 1, :], ps_o0[DH:DH + 1, :])
                    nc.vector.reciprocal(inv1[DH:DH + 1, :], ps_o1[DH:DH + 1, :])
                # broadcast [1, 512] -> [64, 512] via K=1 matmul
                ps_b0 = ps_small.tile([DH, 512], f32, tag="small")
                ps_b1 = ps_small.tile([DH, 512], f32, tag="small")
                nc.tensor.matmul(
                    ps_b0, lhsT=ones_t[DH:DH + 1, :], rhs=inv0[DH:DH + 1, :]
                )
                nc.tensor.matmul(
                    ps_b1, lhsT=ones_t[DH:DH + 1, :], rhs=inv1[DH:DH + 1, :]
                )
                # DVE can read only one PSUM operand per op: stage the
                # broadcast factors in SBUF first.
                b0_sb = work.tile([DH, 512], f32r, tag="bsb")
                b1_sb = work.tile([DH, 512], f32r, tag="bsb")
                nc.vector.tensor_copy(b0_sb, ps_b0)
                nc.vector.tensor_copy(b1_sb, ps_b1)
                # normalized O^T, assembled [128, 512] (h0 rows 0-63,
                # h1 shifted to 64-127 via SBUF->SBUF DMA)
                otall = work.tile([P, 512], f32r, tag="otall")
                tmp1 = work.tile([DH, 512], f32r, tag="tmp1")
                nc.vector.tensor_mul(otall[0:DH, :], ps_o0[0:DH, :], b0_sb)
                nc.vector.tensor_mul(tmp1, ps_o1[0:DH, :], b1_sb)
                nc.sync.dma_start(out=otall[DH:P, :], in_=tmp1)
                # output projection: y[q, :] = O^T[:, q].T @ Wo_pair
                for sub in range(4):
                    ps_y = ps_small.tile([P, 512], f32, tag="small")
                    nc.tensor.matmul(
                        ps_y,
                        lhsT=otall[:, sub * P:(sub + 1) * P],
                        rhs=wo_sb,
                    )
                    ysb = work.tile([P, 512], f32, tag="ysb")
                    nc.vector.tensor_copy(ysb, ps_y)
                    r0 = qc * 512 + sub * P
                    nc.sync.dma_start(out=y[r0:r0 + P, :], in_=ysb)

            for qc in range(QC):
                for kb in range(KB):
                    emit_step(qc, kb)
                    if kb == K_PRE and qc > 0:
                        emit_epilogue(qc - 1)
                    if kb >= 1:
                        emit_pv(qc, kb - 1)
                emit_pv(qc, KB - 1)
            emit_epilogue(QC - 1)

    if not nc.is_finalized():
        nc.finalize()
    return nc


def _get_nc():
    global _CACHED_NC
    if _CACHED_NC is None:
        _CACHED_NC = build_nc()
    return _CACHED_NC


def kernel(x, Wq, Wk, Wv, Wo, bo):
    global LAST_RESULT
    x = np.asarray(x, dtype=np.float32)
    Wq = np.asarray(Wq, dtype=np.float32)
    Wk = np.asarray(Wk, dtype=np.float32)
    Wv = np.asarray(Wv, dtype=np.float32)
    Wo = np.asarray(Wo, dtype=np.float32)
    bo = np.asarray(bo, dtype=np.float32)

    in_maps = []
    for c in range(NCORES):
        b, p = c // 4, c % 4
        cols = slice(p * P, (p + 1) * P)
        in_maps.append({
            "xT": np.ascontiguousarray(x[b].T),
            "wq": np.ascontiguousarray(Wq[:, cols]),
            "wk": np.ascontiguousarray(Wk[:, cols]),
            "wv": np.ascontiguousarray(Wv[:, cols]),
            "wo": np.ascontiguousarray(Wo[cols, :]),
            "ident": np.eye(P, dtype=np.float32),
            "ones": np.ones((P, DH), dtype=np.float32),
        })

    nc = _get_nc()
    res = run_bass_kernel_spmd(nc, in_maps, list(range(NCORES)), trace=TRACE)
    LAST_RESULT = res

    out = np.zeros((B, N, D), dtype=np.float32)
    for c in range(NCORES):
        out[c // 4] += res.results[c]["y"]
    out += bo[None, None, :]
    return out


# revision 21
# speedup vs baseline: 1.0976x; 1.0976x over previous
"""Trainium2 Bass kernel for CrossAttention (self-attention) nn module.

Reference computation (B=2, N=4096, D=512, H=8, DH=64):
    q, k, v = x@Wq, x@Wk, x@Wv          # [B, N, 512]
    per head: S = q k^T / sqrt(64); P = softmax(S); O = P v
    out = concat_heads(O) @ Wo + bo     # [B, N, 512]

Sharding: batch*head-pair across 8 cores. Core c handles batch c//4 and
head pair c%4 (heads 2p, 2p+1). Each core computes its two heads'
attention plus its partial output projection O_pair @ Wo[rows]; the host
sums the four partials per batch and adds the bias.

Device-side strategy (per core):
  - Host supplies x[b]^T so the contraction dim (D) lands on partitions.
  - QT/KT computed head-transposed [128(2 heads x 64), 4096] via fp32r
    matmuls (fp32 inputs), cast to bf16 on the PSUM->SBUF copy.
  - S^T[keys, q] = K @ Q^T per head in bf16 (1 cyc/row), K=64 row-packed:
    head0 in PE rows 0-63, head1 in rows 64-127, running concurrently.
  - exp on ScalarE from PSUM ([128, 1024] tiles covering both heads),
    writing bf16 P^T. No max subtraction (scores are O(1); exp is
    shift-invariant under softmax and fp32 cannot overflow here).
  - PV in bf16 with V in natural layout augmented by a ones column
    (M=65): PSUM row 64 accumulates the softmax denominator for free.
  - normalize: reciprocal_approx_fast on DVE, K=1 broadcast matmul,
    multiply to bf16 O^T per head; output projection accumulates the two
    heads' K=64 bf16 matmuls into one PSUM tile (host packs Wo rows as
    [64, 2, 512] so both heads sit at partitions 0-63).

End-to-end rel err vs fp32 reference: ~7e-4 (bf16 QK/P/V paths).
"""

import os
import sys

import numpy as np

for _p in ("/opt/trn_rl_repo", "/root/.axon_site/_ro/trn_rl_repo"):
    if os.path.isdir(_p) and _p not in sys.path:
        sys.path.insert(0, _p)

import ml_dtypes  # noqa: E402

import concourse.bass as bass  # noqa: E402
import concourse.mybir as mybir  # noqa: E402
from concourse import bacc  # noqa: E402
from concourse.bass_utils import run_bass_kernel_spmd  # noqa: E402
from concourse.tile import TileContext  # noqa: E402

B, N, D = 2, 4096, 512
H, DH = 8, 64
P = 128                 # SBUF partitions / token block
KB = N // P             # 32 key blocks
QC = N // 512           # 8 query column blocks of 512
KCH = D // P            # 4 contraction chunks for the projections
SCALE = DH ** -0.5
NCORES = 8
K_PRE = 4               # S^T/exp steps emitted before prev qc's epilogue

# knobs for test.py
TRACE = False
LAST_RESULT = None

_CACHED_NC = None


def build_nc():
    f32 = mybir.dt.float32
    f32r = mybir.dt.float32r
    bf16 = mybir.dt.bfloat16
    Exp = mybir.ActivationFunctionType.Exp

    nc = bacc.Bacc()
    xT = nc.declare_dram_parameter("xT", [D, N], f32r, isOutput=False)
    wq = nc.declare_dram_parameter("wq", [D, P], f32r, isOutput=False)
    wk = nc.declare_dram_parameter("wk", [D, P], f32r, isOutput=False)
    wv = nc.declare_dram_parameter("wv", [D, P], f32r, isOutput=False)
    wo2_d = nc.declare_dram_parameter("wo2", [DH, 2, D], bf16, isOutput=False)
    ident_d = nc.declare_dram_parameter("ident", [P, P], bf16, isOutput=False)
    ones_d = nc.declare_dram_parameter("ones", [P, DH], f32r, isOutput=False)
    onesbf_d = nc.declare_dram_parameter("onesbf", [P, DH], bf16, isOutput=False)
    y = nc.declare_dram_parameter("y", [N, D], f32, isOutput=True)

    with TileContext(nc) as tc:
        with (
            tc.tile_pool(name="persist", bufs=1) as persist,
            tc.tile_pool(name="proj", bufs=1) as proj,
            tc.tile_pool(name="ptp", bufs=3) as ptp,
            tc.tile_pool(name="work", bufs=3) as work,
            tc.tile_pool(name="ps_big", bufs=2, space="PSUM") as ps_big,
            tc.tile_pool(name="ps_acc", bufs=2, space="PSUM") as ps_acc,
            tc.tile_pool(name="ps_small", bufs=2, space="PSUM") as ps_small,
        ):
            # ---------------- prologue: loads ----------------
            xt_sb = persist.tile([P, KCH, N], f32r, tag="xt")
            for c in range(KCH):
                nc.sync.dma_start(out=xt_sb[:, c, :], in_=xT[c * P:(c + 1) * P, :])

            wq_sb = persist.tile([P, KCH, P], f32r, tag="wq")
            wk_sb = persist.tile([P, KCH, P], f32r, tag="wk")
            wv_sb = persist.tile([P, KCH, P], f32r, tag="wv")
            for w_sb, w_d in ((wq_sb, wq), (wk_sb, wk), (wv_sb, wv)):
                nc.sync.dma_start(
                    out=w_sb, in_=w_d.rearrange("(c p) m -> p c m", p=P)
                )
            wo2_sb = persist.tile([DH, 2, D], bf16, tag="wo2")
            nc.sync.dma_start(out=wo2_sb, in_=wo2_d[:, :, :])

            ident = persist.tile([P, P], bf16, tag="ident")
            nc.sync.dma_start(out=ident, in_=ident_d[:, :])
            ones_t = persist.tile([P, DH], f32r, tag="ones")
            nc.sync.dma_start(out=ones_t, in_=ones_d[:, :])

            # ---------------- projections ----------------
            # QT/KT/VT: [128 (2 heads x 64 dims), 4096 tokens], bf16
            qt = persist.tile([P, N], bf16, tag="qt")
            kt = persist.tile([P, N], bf16, tag="kt")
            vt = proj.tile([P, N], bf16, tag="vt")
            for dst, w_sb in ((qt, wq_sb), (kt, wk_sb), (vt, wv_sb)):
                for col in range(QC):
                    csl = slice(col * 512, (col + 1) * 512)
                    psp = ps_small.tile([P, 512], f32, tag="small")
                    for c in range(KCH):
                        nc.tensor.matmul(
                            psp,
                            lhsT=w_sb[:, c, :],
                            rhs=xt_sb[:, c, csl],
                            start=(c == 0),
                            stop=(c == KCH - 1),
                        )
                    nc.vector.tensor_copy(dst[:, csl], psp)

            # V natural layout + ones column: v_aug[:, kb, h, 0:64] = V block,
            # v_aug[:, kb, h, 64] = 1.0
            v_aug = persist.tile([P, KB, 2, DH + 1], bf16, tag="vaug")
            nc.sync.dma_start(
                out=v_aug[:, :, :, DH:DH + 1],
                in_=onesbf_d[:, :].rearrange("p (a b c) -> p a b c", a=KB, b=2),
            )
            for kb in range(KB):
                pst = ps_small.tile([P, P], bf16, tag="small")
                nc.tensor.transpose(pst, vt[:, kb * P:(kb + 1) * P], ident)
                nc.vector.tensor_copy(
                    v_aug[:, kb, :, 0:DH],
                    pst.rearrange("p (h d) -> p h d", h=2),
                )

            # ---------------- attention + output projection ----------------
            state = {}

            def emit_step(qc, kb):
                """S^T for both heads (row-packed) + exp."""
                qsl = slice(qc * 512, (qc + 1) * 512)
                ksl = slice(kb * P, (kb + 1) * P)
                ps_s = ps_big.tile([P, 1024], f32, tag="psS")
                nc.tensor.matmul(
                    ps_s[:, 0:512],
                    lhsT=kt[0:DH, ksl],
                    rhs=qt[0:DH, qsl],
                )
                nc.tensor.matmul(
                    ps_s[:, 512:1024],
                    lhsT=kt[DH:P, ksl],
                    rhs=qt[DH:P, qsl],
                )
                pt = ptp.tile([P, 1024], bf16, tag="pt")
                nc.scalar.activation(pt, ps_s, func=Exp, scale=SCALE)
                state[(qc, kb)] = pt

            def emit_pv(qc, kb):
                pt = state.pop((qc, kb))
                if kb == 0:
                    state[(qc, "o0")] = ps_acc.tile(
                        [DH + 1, 512], f32, tag="psO", name="ps_o0"
                    )
                    state[(qc, "o1")] = ps_acc.tile(
                        [DH + 1, 512], f32, tag="psO", name="ps_o1"
                    )
                for h in range(2):
                    nc.tensor.matmul(
                        state[(qc, "o0" if h == 0 else "o1")],
                        lhsT=v_aug[:, kb, h, :],
                        rhs=pt[:, h * 512:(h + 1) * 512],
                        start=(kb == 0),
                        stop=(kb == KB - 1),
                    )

            def emit_epilogue(qc):
                ps_o0 = state.pop((qc, "o0"))
                ps_o1 = state.pop((qc, "o1"))
                # softmax sums (PSUM row 64) -> SBUF row 64, then broadcast
                # [1, 512] -> [64, 512] via K=1 matmul, THEN reciprocal.
                # (reciprocal_approx_fast silently returns zeros when fed
                # PSUM on HW, so it must run SBUF->SBUF after the bcast.)
                s_sb0 = work.tile([DH + 1, 512], f32r, tag="ssb")
                s_sb1 = work.tile([DH + 1, 512], f32r, tag="ssb")
                nc.vector.tensor_copy(s_sb0[DH:DH + 1, :], ps_o0[DH:DH + 1, :])
                nc.vector.tensor_copy(s_sb1[DH:DH + 1, :], ps_o1[DH:DH + 1, :])
                ps_b0 = ps_small.tile([DH, 512], f32, tag="small")
                ps_b1 = ps_small.tile([DH, 512], f32, tag="small")
                nc.tensor.matmul(
                    ps_b0, lhsT=ones_t[DH:DH + 1, :], rhs=s_sb0[DH:DH + 1, :]
                )
                nc.tensor.matmul(
                    ps_b1, lhsT=ones_t[DH:DH + 1, :], rhs=s_sb1[DH:DH + 1, :]
                )
                b0_sb = work.tile([DH, 512], f32, tag="bsb")
                b1_sb = work.tile([DH, 512], f32, tag="bsb")
                nc.vector.tensor_copy(b0_sb, ps_b0)
                nc.vector.tensor_copy(b1_sb, ps_b1)
                binv0 = work.tile([DH, 512], f32, tag="binv")
                binv1 = work.tile([DH, 512], f32, tag="binv")
                nc.vector.reciprocal_approx_fast(out=binv0, in_=b0_sb)
                nc.vector.reciprocal_approx_fast(out=binv1, in_=b1_sb)
                # normalized per-head O^T in bf16 (both at partitions 0-63)
                otn0 = work.tile([DH, 512], bf16, tag="otn")
                otn1 = work.tile([DH, 512], bf16, tag="otn")
                nc.vector.tensor_mul(otn0, ps_o0[0:DH, :], binv0)
                nc.vector.tensor_mul(otn1, ps_o1[0:DH, :], binv1)
                # output projection: y[q, :] = sum_h O_h[q, :] @ Wo_h
                for sub in range(4):
                    ssl = slice(sub * P, (sub + 1) * P)
                    ps_y = ps_small.tile([P, 512], f32, tag="small")
                    nc.tensor.matmul(
                        ps_y, lhsT=otn0[:, ssl], rhs=wo2_sb[:, 0, :],
                        start=True, stop=False,
                    )
                    nc.tensor.matmul(
                        ps_y, lhsT=otn1[:, ssl], rhs=wo2_sb[:, 1, :],
                        start=False, stop=True,
                    )
                    ysb = work.tile([P, 512], f32, tag="ysb")
                    nc.vector.tensor_copy(ysb, ps_y)
                    r0 = qc * 512 + sub * P
                    nc.sync.dma_start(out=y[r0:r0 + P, :], in_=ysb)

            for qc in range(QC):
                for kb in range(KB):
                    emit_step(qc, kb)
                    if kb == K_PRE and qc > 0:
                        emit_epilogue(qc - 1)
                    if kb >= 1:
                        emit_pv(qc, kb - 1)
                emit_pv(qc, KB - 1)
            emit_epilogue(QC - 1)

    if not nc.is_finalized():
        nc.finalize()
    return nc


def _get_nc():
    global _CACHED_NC
    if _CACHED_NC is None:
        _CACHED_NC = build_nc()
    return _CACHED_NC


def make_in_maps(x, Wq, Wk, Wv, Wo):
    in_maps = []
    for c in range(NCORES):
        b, p = c // 4, c % 4
        cols = slice(p * P, (p + 1) * P)
        wo2 = (
            Wo[cols, :]
            .reshape(2, DH, D)
            .transpose(1, 0, 2)
            .astype(ml_dtypes.bfloat16)
        )
        in_maps.append({
            "xT": np.ascontiguousarray(x[b].T),
            "wq": np.ascontiguousarray(Wq[:, cols]),
            "wk": np.ascontiguousarray(Wk[:, cols]),
            "wv": np.ascontiguousarray(Wv[:, cols]),
            "wo2": np.ascontiguousarray(wo2),
            "ident": np.eye(P, dtype=ml_dtypes.bfloat16),
            "ones": np.ones((P, DH), dtype=np.float32),  # f32r param, same bits
            "onesbf": np.ones((P, DH), dtype=ml_dtypes.bfloat16),
        })
    return in_maps


def kernel(x, Wq, Wk, Wv, Wo, bo):
    global LAST_RESULT
    x = np.asarray(x, dtype=np.float32)
    Wq = np.asarray(Wq, dtype=np.float32)
    Wk = np.asarray(Wk, dtype=np.float32)
    Wv = np.asarray(Wv, dtype=np.float32)
    Wo = np.asarray(Wo, dtype=np.float32)
    bo = np.asarray(bo, dtype=np.float32)

    in_maps = make_in_maps(x, Wq, Wk, Wv, Wo)
    nc = _get_nc()
    res = run_bass_kernel_spmd(nc, in_maps, list(range(NCORES)), trace=TRACE)
    LAST_RESULT = res

    out = np.zeros((B, N, D), dtype=np.float32)
    for c in range(NCORES):
        out[c // 4] += res.results[c]["y"]
    out += bo[None, None, :]
    return out


# revision 25
# speedup vs baseline: 1.2453x; 1.1346x over previous
"""Trainium2 Bass kernel for CrossAttention (self-attention) nn module.

Reference computation (B=2, N=4096, D=512, H=8, DH=64):
    q, k, v = x@Wq, x@Wk, x@Wv          # [B, N, 512]
    per head: S = q k^T / sqrt(64); P = softmax(S); O = P v
    out = concat_heads(O) @ Wo + bo     # [B, N, 512]

Sharding: batch*head-pair across 8 cores. Core c handles batch c//4 and
head pair c%4 (heads 2p, 2p+1). Each core computes its two heads'
attention plus its partial output projection O_pair @ Wo[rows]; the host
sums the four partials per batch and adds the bias.

Device-side strategy (per core):
  - Host supplies x[b]^T so the contraction dim (D) lands on partitions.
  - QT/KT computed head-transposed [128(2 heads x 64), 4096] via fp32r
    matmuls (fp32 inputs), cast to bf16 on the PSUM->SBUF copy.
  - S^T[keys, q] = K @ Q^T per head in bf16 (1 cyc/row), K=64 row-packed:
    head0 in PE rows 0-63, head1 in rows 64-127, running concurrently.
  - exp on ScalarE from PSUM ([128, 1024] tiles covering both heads),
    writing bf16 P^T. No max subtraction (scores are O(1); exp is
    shift-invariant under softmax and fp32 cannot overflow here).
  - PV in bf16 with V in natural layout augmented by a ones column
    (M=65): PSUM row 64 accumulates the softmax denominator for free.
  - normalize: reciprocal_approx_fast on DVE, K=1 broadcast matmul,
    multiply to bf16 O^T per head; output projection accumulates the two
    heads' K=64 bf16 matmuls into one PSUM tile (host packs Wo rows as
    [64, 2, 512] so both heads sit at partitions 0-63).

End-to-end rel err vs fp32 reference: ~7e-4 (bf16 QK/P/V paths).
"""

import os
import sys

import numpy as np

for _p in ("/opt/trn_rl_repo", "/root/.axon_site/_ro/trn_rl_repo"):
    if os.path.isdir(_p) and _p not in sys.path:
        sys.path.insert(0, _p)

import ml_dtypes  # noqa: E402

import concourse.bass as bass  # noqa: E402
import concourse.mybir as mybir  # noqa: E402
from concourse import bacc  # noqa: E402
from concourse.bass_utils import run_bass_kernel_spmd  # noqa: E402
from concourse.tile import TileContext  # noqa: E402

B, N, D = 2, 4096, 512
H, DH = 8, 64
P = 128                 # SBUF partitions / token block
KB = N // P             # 32 key blocks
QC = N // 512           # 8 query column blocks of 512
KCH = D // P            # 4 contraction chunks for the projections
SCALE = DH ** -0.5
NCORES = 8
K_PRE = 4               # S^T/exp steps emitted before prev qc's epilogue

# knobs for test.py
TRACE = False
LAST_RESULT = None

_CACHED_NC = None


def build_nc():
    f32 = mybir.dt.float32
    f32r = mybir.dt.float32r
    bf16 = mybir.dt.bfloat16
    Exp = mybir.ActivationFunctionType.Exp

    nc = bacc.Bacc()
    xT = nc.declare_dram_parameter("xT", [D, N], f32r, isOutput=False)
    wq = nc.declare_dram_parameter("wq", [D, P], f32r, isOutput=False)
    wk = nc.declare_dram_parameter("wk", [D, P], f32r, isOutput=False)
    wv = nc.declare_dram_parameter("wv", [D, P], f32r, isOutput=False)
    wo2_d = nc.declare_dram_parameter("wo2", [DH, 2, D], bf16, isOutput=False)
    ident_d = nc.declare_dram_parameter("ident", [P, P], bf16, isOutput=False)
    ones_d = nc.declare_dram_parameter("ones", [P, DH], f32r, isOutput=False)
    onesbf_d = nc.declare_dram_parameter("onesbf", [P, DH], bf16, isOutput=False)
    y = nc.declare_dram_parameter("y", [N, D], f32, isOutput=True)

    with TileContext(nc) as tc:
        with (
            tc.tile_pool(name="persist", bufs=1) as persist,
            tc.tile_pool(name="proj", bufs=1) as proj,
            tc.tile_pool(name="ptp", bufs=4) as ptp,
            tc.tile_pool(name="work", bufs=3) as work,
            tc.tile_pool(name="ps_big", bufs=2, space="PSUM") as ps_big,
            tc.tile_pool(name="ps_acc", bufs=2, space="PSUM") as ps_acc,
            tc.tile_pool(name="ps_small", bufs=2, space="PSUM") as ps_small,
        ):
            # ---------------- prologue: loads ----------------
            xt_sb = persist.tile([P, KCH, N], f32r, tag="xt")
            for c in range(KCH):
                nc.sync.dma_start(out=xt_sb[:, c, :], in_=xT[c * P:(c + 1) * P, :])

            wq_sb = persist.tile([P, KCH, P], f32r, tag="wq")
            wk_sb = persist.tile([P, KCH, P], f32r, tag="wk")
            wv_sb = persist.tile([P, KCH, P], f32r, tag="wv")
            for w_sb, w_d in ((wq_sb, wq), (wk_sb, wk), (wv_sb, wv)):
                nc.sync.dma_start(
                    out=w_sb, in_=w_d.rearrange("(c p) m -> p c m", p=P)
                )
            wo2_sb = persist.tile([DH, 2, D], bf16, tag="wo2")
            nc.sync.dma_start(out=wo2_sb, in_=wo2_d[:, :, :])

            ident = persist.tile([P, P], bf16, tag="ident")
            nc.sync.dma_start(out=ident, in_=ident_d[:, :])
            ones_t = persist.tile([P, DH], f32r, tag="ones")
            nc.sync.dma_start(out=ones_t, in_=ones_d[:, :])

            # ---------------- projections ----------------
            # QT/KT/VT: [128 (2 heads x 64 dims), 4096 tokens], bf16
            qt = persist.tile([P, N], bf16, tag="qt")
            kt = persist.tile([P, N], bf16, tag="kt")
            vt = proj.tile([P, N], bf16, tag="vt")
            for dst, w_sb in ((qt, wq_sb), (kt, wk_sb), (vt, wv_sb)):
                for col in range(QC):
                    csl = slice(col * 512, (col + 1) * 512)
                    psp = ps_small.tile([P, 512], f32, tag="small")
                    for c in range(KCH):
                        nc.tensor.matmul(
                            psp,
                            lhsT=w_sb[:, c, :],
                            rhs=xt_sb[:, c, csl],
                            start=(c == 0),
                            stop=(c == KCH - 1),
                        )
                    nc.vector.tensor_copy(dst[:, csl], psp)

            # V natural layout + ones column: v_aug[:, kb, h, 0:64] = V block,
            # v_aug[:, kb, h, 64] = 1.0
            v_aug = persist.tile([P, KB, 2, DH + 1], bf16, tag="vaug")
            nc.sync.dma_start(
                out=v_aug[:, :, :, DH:DH + 1],
                in_=onesbf_d[:, :].rearrange("p (a b c) -> p a b c", a=KB, b=2),
            )
            for kb in range(KB):
                pst = ps_small.tile([P, P], bf16, tag="small")
                nc.tensor.transpose(pst, vt[:, kb * P:(kb + 1) * P], ident)
                nc.vector.tensor_copy(
                    v_aug[:, kb, :, 0:DH],
                    pst.rearrange("p (h d) -> p h d", h=2),
                )

            # ---------------- attention + output projection ----------------
            state = {}

            def emit_step(qc, kb):
                """S^T for both heads (row-packed) + exp."""
                qsl = slice(qc * 512, (qc + 1) * 512)
                ksl = slice(kb * P, (kb + 1) * P)
                ps_s = ps_big.tile([P, 1024], f32, tag="psS")
                nc.tensor.matmul(
                    ps_s[:, 0:512],
                    lhsT=kt[0:DH, ksl],
                    rhs=qt[0:DH, qsl],
                )
                nc.tensor.matmul(
                    ps_s[:, 512:1024],
                    lhsT=kt[DH:P, ksl],
                    rhs=qt[DH:P, qsl],
                )
                pt = ptp.tile([P, 1024], bf16, tag="pt")
                nc.scalar.activation(pt, ps_s, func=Exp, scale=SCALE)
                state[(qc, kb)] = pt

            def emit_pv(qc, kb):
                pt = state.pop((qc, kb))
                if kb == 0:
                    state[(qc, "o0")] = ps_acc.tile(
                        [DH + 1, 512], f32, tag="psO", name="ps_o0"
                    )
                    state[(qc, "o1")] = ps_acc.tile(
                        [DH + 1, 512], f32, tag="psO", name="ps_o1"
                    )
                for h in range(2):
                    nc.tensor.matmul(
                        state[(qc, "o0" if h == 0 else "o1")],
                        lhsT=v_aug[:, kb, h, :],
                        rhs=pt[:, h * 512:(h + 1) * 512],
                        start=(kb == 0),
                        stop=(kb == KB - 1),
                    )

            def emit_norm(qc):
                ps_o0 = state.pop((qc, "o0"))
                ps_o1 = state.pop((qc, "o1"))
                # softmax sums (PSUM row 64) -> SBUF row 64, then broadcast
                # [1, 512] -> [64, 512] via K=1 matmul, THEN reciprocal.
                # (reciprocal_approx_fast silently returns zeros when fed
                # PSUM on HW, so it must run SBUF->SBUF after the bcast.)
                s_sb0 = work.tile([DH + 1, 512], f32r, tag="ssb")
                s_sb1 = work.tile([DH + 1, 512], f32r, tag="ssb")
                nc.vector.tensor_copy(s_sb0[DH:DH + 1, :], ps_o0[DH:DH + 1, :])
                nc.vector.tensor_copy(s_sb1[DH:DH + 1, :], ps_o1[DH:DH + 1, :])
                ps_b0 = ps_small.tile([DH, 512], f32, tag="small")
                ps_b1 = ps_small.tile([DH, 512], f32, tag="small")
                nc.tensor.matmul(
                    ps_b0, lhsT=ones_t[DH:DH + 1, :], rhs=s_sb0[DH:DH + 1, :]
                )
                nc.tensor.matmul(
                    ps_b1, lhsT=ones_t[DH:DH + 1, :], rhs=s_sb1[DH:DH + 1, :]
                )
                b0_sb = work.tile([DH, 512], f32, tag="bsb")
                b1_sb = work.tile([DH, 512], f32, tag="bsb")
                nc.vector.tensor_copy(b0_sb, ps_b0)
                nc.vector.tensor_copy(b1_sb, ps_b1)
                binv0 = work.tile([DH, 512], f32, tag="binv")
                binv1 = work.tile([DH, 512], f32, tag="binv")
                nc.vector.reciprocal_approx_fast(out=binv0, in_=b0_sb)
                nc.vector.reciprocal_approx_fast(out=binv1, in_=b1_sb)
                # normalized per-head O^T in bf16 (both at partitions 0-63)
                otn0 = work.tile([DH, 512], bf16, tag="otn")
                otn1 = work.tile([DH, 512], bf16, tag="otn")
                nc.vector.tensor_mul(otn0, ps_o0[0:DH, :], binv0)
                nc.vector.tensor_mul(otn1, ps_o1[0:DH, :], binv1)
                state[(qc, "otn")] = (otn0, otn1)

            def emit_proj(qc):
                # output projection: y[q, :] = sum_h O_h[q, :] @ Wo_h
                otn0, otn1 = state.pop((qc, "otn"))
                for sub in range(4):
                    ssl = slice(sub * P, (sub + 1) * P)
                    ps_y = ps_small.tile([P, 512], f32, tag="small")
                    nc.tensor.matmul(
                        ps_y, lhsT=otn0[:, ssl], rhs=wo2_sb[:, 0, :],
                        start=True, stop=False,
                    )
                    nc.tensor.matmul(
                        ps_y, lhsT=otn1[:, ssl], rhs=wo2_sb[:, 1, :],
                        start=False, stop=True,
                    )
                    ysb = work.tile([P, 512], f32, tag="ysb")
                    nc.vector.tensor_copy(ysb, ps_y)
                    r0 = qc * 512 + sub * P
                    nc.sync.dma_start(out=y[r0:r0 + P, :], in_=ysb)

            # Software pipeline: PV lags S^T/exp by 2 steps; the previous
            # qc's normalize chain is emitted right after its last PV (PE
            # content: just two K=1 bcasts), and its output projection six
            # steps into the next qc, when the DVE chain has long drained.
            # This keeps the PE stream dense across qc boundaries so HAM
            # never re-throttles.
            for qc in range(QC):
                for kb in range(KB):
                    emit_step(qc, kb)
                    if qc > 0:
                        if kb == 0:
                            emit_norm(qc - 1)
                        elif kb == 6:
                            emit_proj(qc - 1)
                    if kb >= 2:
                        emit_pv(qc, kb - 2)
                emit_pv(qc, KB - 2)
                emit_pv(qc, KB - 1)
            emit_norm(QC - 1)
            emit_proj(QC - 1)

    if not nc.is_finalized():
        nc.finalize()
    return nc


def _get_nc():
    global _CACHED_NC
    if _CACHED_NC is None:
        _CACHED_NC = build_nc()
    return _CACHED_NC


def make_in_maps(x, Wq, Wk, Wv, Wo):
    in_maps = []
    for c in range(NCORES):
        b, p = c // 4, c % 4
        cols = slice(p * P, (p + 1) * P)
        wo2 = (
            Wo[cols, :]
            .reshape(2, DH, D)
            .transpose(1, 0, 2)
            .astype(ml_dtypes.bfloat16)
        )
        in_maps.append({
            "xT": np.ascontiguousarray(x[b].T),
            "wq": np.ascontiguousarray(Wq[:, cols]),
            "wk": np.ascontiguousarray(Wk[:, cols]),
            "wv": np.ascontiguousarray(Wv[:, cols]),
            "wo2": np.ascontiguousarray(wo2),
            "ident": np.eye(P, dtype=ml_dtypes.bfloat16),
            "ones": np.ones((P, DH), dtype=np.float32),  # f32r param, same bits
            "onesbf": np.ones((P, DH), dtype=ml_dtypes.bfloat16),
        })
    return in_maps


def kernel(x, Wq, Wk, Wv, Wo, bo):
    global LAST_RESULT
    x = np.asarray(x, dtype=np.float32)
    Wq = np.asarray(Wq, dtype=np.float32)
    Wk = np.asarray(Wk, dtype=np.float32)
    Wv = np.asarray(Wv, dtype=np.float32)
    Wo = np.asarray(Wo, dtype=np.float32)
    bo = np.asarray(bo, dtype=np.float32)

    in_maps = make_in_maps(x, Wq, Wk, Wv, Wo)
    nc = _get_nc()
    res = run_bass_kernel_spmd(nc, in_maps, list(range(NCORES)), trace=TRACE)
    LAST_RESULT = res

    out = np.zeros((B, N, D), dtype=np.float32)
    for c in range(NCORES):
        out[c // 4] += res.results[c]["y"]
    out += bo[None, None, :]
    return out


# revision 28
# speedup vs baseline: 1.2864x; 1.0330x over previous
"""Trainium2 Bass kernel for CrossAttention (self-attention) nn module.

Reference computation (B=2, N=4096, D=512, H=8, DH=64):
    q, k, v = x@Wq, x@Wk, x@Wv          # [B, N, 512]
    per head: S = q k^T / sqrt(64); P = softmax(S); O = P v
    out = concat_heads(O) @ Wo + bo     # [B, N, 512]

Sharding: batch*head-pair across 8 cores. Core c handles batch c//4 and
head pair c%4 (heads 2p, 2p+1). Each core computes its two heads'
attention plus its partial output projection O_pair @ Wo[rows]; the host
sums the four partials per batch and adds the bias.

Device-side strategy (per core):
  - Host supplies x[b]^T so the contraction dim (D) lands on partitions.
  - QT/KT computed head-transposed [128(2 heads x 64), 4096] via fp32r
    matmuls (fp32 inputs), cast to bf16 on the PSUM->SBUF copy.
  - S^T[keys, q] = K @ Q^T per head in bf16 (1 cyc/row), K=64 row-packed:
    head0 in PE rows 0-63, head1 in rows 64-127, running concurrently.
  - exp on ScalarE from PSUM ([128, 1024] tiles covering both heads),
    writing bf16 P^T. No max subtraction (scores are O(1); exp is
    shift-invariant under softmax and fp32 cannot overflow here).
  - PV in bf16 with V in natural layout augmented by a ones column
    (M=65): PSUM row 64 accumulates the softmax denominator for free.
  - normalize: reciprocal_approx_fast on DVE, K=1 broadcast matmul,
    multiply to bf16 O^T per head; output projection accumulates the two
    heads' K=64 bf16 matmuls into one PSUM tile (host packs Wo rows as
    [64, 2, 512] so both heads sit at partitions 0-63).

End-to-end rel err vs fp32 reference: ~7e-4 (bf16 QK/P/V paths).
"""

import os
import sys

import numpy as np

for _p in ("/opt/trn_rl_repo", "/root/.axon_site/_ro/trn_rl_repo"):
    if os.path.isdir(_p) and _p not in sys.path:
        sys.path.insert(0, _p)

import ml_dtypes  # noqa: E402

import concourse.bass as bass  # noqa: E402
import concourse.mybir as mybir  # noqa: E402
from concourse import bacc  # noqa: E402
from concourse.bass_utils import run_bass_kernel_spmd  # noqa: E402
from concourse.tile import TileContext  # noqa: E402

B, N, D = 2, 4096, 512
H, DH = 8, 64
P = 128                 # SBUF partitions / token block
KB = N // P             # 32 key blocks
QC = N // 512           # 8 query column blocks of 512
KCH = D // P            # 4 contraction chunks for the projections
SCALE = DH ** -0.5
NCORES = 8
K_PRE = 4               # S^T/exp steps emitted before prev qc's epilogue

# knobs for test.py
TRACE = False
LAST_RESULT = None

_CACHED_NC = None


def build_nc():
    f32 = mybir.dt.float32
    f32r = mybir.dt.float32r
    bf16 = mybir.dt.bfloat16
    Exp = mybir.ActivationFunctionType.Exp

    nc = bacc.Bacc()
    xT = nc.declare_dram_parameter("xT", [D, N], bf16, isOutput=False)
    wq = nc.declare_dram_parameter("wq", [D, P], bf16, isOutput=False)
    wk = nc.declare_dram_parameter("wk", [D, P], bf16, isOutput=False)
    wv = nc.declare_dram_parameter("wv", [D, P], bf16, isOutput=False)
    wo2_d = nc.declare_dram_parameter("wo2", [DH, 2, D], bf16, isOutput=False)
    ident_d = nc.declare_dram_parameter("ident", [P, P], bf16, isOutput=False)
    ones_d = nc.declare_dram_parameter("ones", [P, DH], f32r, isOutput=False)
    onesbf_d = nc.declare_dram_parameter("onesbf", [P, DH], bf16, isOutput=False)
    y = nc.declare_dram_parameter("y", [N, D], f32, isOutput=True)

    with TileContext(nc) as tc:
        with (
            tc.tile_pool(name="persist", bufs=1) as persist,
            tc.tile_pool(name="proj", bufs=1) as proj,
            tc.tile_pool(name="ptp", bufs=4) as ptp,
            tc.tile_pool(name="work", bufs=3) as work,
            tc.tile_pool(name="ps_big", bufs=2, space="PSUM") as ps_big,
            tc.tile_pool(name="ps_acc", bufs=2, space="PSUM") as ps_acc,
            tc.tile_pool(name="ps_small", bufs=2, space="PSUM") as ps_small,
        ):
            # ---------------- prologue: loads ----------------
            xt_sb = persist.tile([P, KCH, N], bf16, tag="xt")
            for c in range(KCH):
                for cc in range(4):
                    csl = slice(cc * 1024, (cc + 1) * 1024)
                    nc.sync.dma_start(
                        out=xt_sb[:, c, csl], in_=xT[c * P:(c + 1) * P, csl]
                    )

            wq_sb = persist.tile([P, KCH, P], bf16, tag="wq")
            wk_sb = persist.tile([P, KCH, P], bf16, tag="wk")
            wv_sb = persist.tile([P, KCH, P], bf16, tag="wv")
            for w_sb, w_d in ((wq_sb, wq), (wk_sb, wk), (wv_sb, wv)):
                nc.sync.dma_start(
                    out=w_sb, in_=w_d.rearrange("(c p) m -> p c m", p=P)
                )
            wo2_sb = persist.tile([DH, 2, D], bf16, tag="wo2")
            nc.sync.dma_start(out=wo2_sb, in_=wo2_d[:, :, :])

            ident = persist.tile([P, P], bf16, tag="ident")
            nc.sync.dma_start(out=ident, in_=ident_d[:, :])
            ones_t = persist.tile([P, DH], f32r, tag="ones")
            nc.sync.dma_start(out=ones_t, in_=ones_d[:, :])

            # ---------------- projections ----------------
            # QT/KT/VT: [128 (2 heads x 64 dims), 4096 tokens], bf16
            qt = persist.tile([P, N], bf16, tag="qt")
            kt = persist.tile([P, N], bf16, tag="kt")
            vt = proj.tile([P, N], bf16, tag="vt")
            for dst, w_sb in ((qt, wq_sb), (kt, wk_sb), (vt, wv_sb)):
                for col in range(QC):
                    csl = slice(col * 512, (col + 1) * 512)
                    psp = ps_small.tile([P, 512], f32, tag="small")
                    for c in range(KCH):
                        nc.tensor.matmul(
                            psp,
                            lhsT=w_sb[:, c, :],
                            rhs=xt_sb[:, c, csl],
                            start=(c == 0),
                            stop=(c == KCH - 1),
                        )
                    nc.vector.tensor_copy(dst[:, csl], psp)

            # V natural layout + ones column: v_aug[:, kb, h, 0:64] = V block,
            # v_aug[:, kb, h, 64] = 1.0
            v_aug = persist.tile([P, KB, 2, DH + 1], bf16, tag="vaug")
            nc.sync.dma_start(
                out=v_aug[:, :, :, DH:DH + 1],
                in_=onesbf_d[:, :].rearrange("p (a b c) -> p a b c", a=KB, b=2),
            )
            for kb in range(KB):
                pst = ps_small.tile([P, P], bf16, tag="small")
                nc.tensor.transpose(pst, vt[:, kb * P:(kb + 1) * P], ident)
                nc.vector.tensor_copy(
                    v_aug[:, kb, :, 0:DH],
                    pst.rearrange("p (h d) -> p h d", h=2),
                )

            # ---------------- attention + output projection ----------------
            state = {}

            def emit_step(qc, kb):
                """S^T for both heads (row-packed) + exp."""
                qsl = slice(qc * 512, (qc + 1) * 512)
                ksl = slice(kb * P, (kb + 1) * P)
                ps_s = ps_big.tile([P, 1024], f32, tag="psS")
                nc.tensor.matmul(
                    ps_s[:, 0:512],
                    lhsT=kt[0:DH, ksl],
                    rhs=qt[0:DH, qsl],
                )
                nc.tensor.matmul(
                    ps_s[:, 512:1024],
                    lhsT=kt[DH:P, ksl],
                    rhs=qt[DH:P, qsl],
                )
                pt = ptp.tile([P, 1024], bf16, tag="pt")
                nc.scalar.activation(pt, ps_s, func=Exp, scale=SCALE)
                state[(qc, kb)] = pt

            def emit_pv(qc, kb):
                pt = state.pop((qc, kb))
                if kb == 0:
                    state[(qc, "o0")] = ps_acc.tile(
                        [DH + 1, 512], f32, tag="psO", name="ps_o0"
                    )
                    state[(qc, "o1")] = ps_acc.tile(
                        [DH + 1, 512], f32, tag="psO", name="ps_o1"
                    )
                for h in range(2):
                    nc.tensor.matmul(
                        state[(qc, "o0" if h == 0 else "o1")],
                        lhsT=v_aug[:, kb, h, :],
                        rhs=pt[:, h * 512:(h + 1) * 512],
                        start=(kb == 0),
                        stop=(kb == KB - 1),
                    )

            def emit_norm(qc):
                ps_o0 = state.pop((qc, "o0"))
                ps_o1 = state.pop((qc, "o1"))
                # softmax sums (PSUM row 64) -> SBUF row 64, then broadcast
                # [1, 512] -> [64, 512] via K=1 matmul, THEN reciprocal.
                # (reciprocal_approx_fast silently returns zeros when fed
                # PSUM on HW, so it must run SBUF->SBUF after the bcast.)
                s_sb0 = work.tile([DH + 1, 512], f32r, tag="ssb")
                s_sb1 = work.tile([DH + 1, 512], f32r, tag="ssb")
                nc.vector.tensor_copy(s_sb0[DH:DH + 1, :], ps_o0[DH:DH + 1, :])
                nc.vector.tensor_copy(s_sb1[DH:DH + 1, :], ps_o1[DH:DH + 1, :])
                ps_b0 = ps_small.tile([DH, 512], f32, tag="small")
                ps_b1 = ps_small.tile([DH, 512], f32, tag="small")
                nc.tensor.matmul(
                    ps_b0, lhsT=ones_t[DH:DH + 1, :], rhs=s_sb0[DH:DH + 1, :]
                )
                nc.tensor.matmul(
                    ps_b1, lhsT=ones_t[DH:DH + 1, :], rhs=s_sb1[DH:DH + 1, :]
                )
                b0_sb = work.tile([DH, 512], f32, tag="bsb")
                b1_sb = work.tile([DH, 512], f32, tag="bsb")
                nc.vector.tensor_copy(b0_sb, ps_b0)
                nc.vector.tensor_copy(b1_sb, ps_b1)
                binv0 = work.tile([DH, 512], f32, tag="binv")
                binv1 = work.tile([DH, 512], f32, tag="binv")
                nc.vector.reciprocal_approx_fast(out=binv0, in_=b0_sb)
                nc.vector.reciprocal_approx_fast(out=binv1, in_=b1_sb)
                # normalized per-head O^T in bf16 (both at partitions 0-63)
                otn0 = work.tile([DH, 512], bf16, tag="otn")
                otn1 = work.tile([DH, 512], bf16, tag="otn")
                nc.vector.tensor_mul(otn0, ps_o0[0:DH, :], binv0)
                nc.vector.tensor_mul(otn1, ps_o1[0:DH, :], binv1)
                state[(qc, "otn")] = (otn0, otn1)

            def emit_proj(qc):
                # output projection: y[q, :] = sum_h O_h[q, :] @ Wo_h
                otn0, otn1 = state.pop((qc, "otn"))
                for sub in range(4):
                    ssl = slice(sub * P, (sub + 1) * P)
                    ps_y = ps_small.tile([P, 512], f32, tag="small")
                    nc.tensor.matmul(
                        ps_y, lhsT=otn0[:, ssl], rhs=wo2_sb[:, 0, :],
                        start=True, stop=False,
                    )
                    nc.tensor.matmul(
                        ps_y, lhsT=otn1[:, ssl], rhs=wo2_sb[:, 1, :],
                        start=False, stop=True,
                    )
                    ysb = work.tile([P, 512], f32, tag="ysb")
                    nc.vector.tensor_copy(ysb, ps_y)
                    r0 = qc * 512 + sub * P
                    nc.sync.dma_start(out=y[r0:r0 + P, :], in_=ysb)

            # Software pipeline: PV lags S^T/exp by 2 steps; the previous
            # qc's normalize chain is emitted right after its last PV (PE
            # content: just two K=1 bcasts), and its output projection six
            # steps into the next qc, when the DVE chain has long drained.
            # This keeps the PE stream dense across qc boundaries so HAM
            # never re-throttles.
            for qc in range(QC):
                for kb in range(KB):
                    emit_step(qc, kb)
                    if qc > 0:
                        if kb == 0:
                            emit_norm(qc - 1)
                        elif kb == 6:
                            emit_proj(qc - 1)
                    if kb >= 2:
                        emit_pv(qc, kb - 2)
                emit_pv(qc, KB - 2)
                emit_pv(qc, KB - 1)
            emit_norm(QC - 1)
            emit_proj(QC - 1)

    if not nc.is_finalized():
        nc.finalize()
    return nc


def _get_nc():
    global _CACHED_NC
    if _CACHED_NC is None:
        _CACHED_NC = build_nc()
    return _CACHED_NC


def make_in_maps(x, Wq, Wk, Wv, Wo):
    in_maps = []
    for c in range(NCORES):
        b, p = c // 4, c % 4
        cols = slice(p * P, (p + 1) * P)
        wo2 = (
            Wo[cols, :]
            .reshape(2, DH, D)
            .transpose(1, 0, 2)
            .astype(ml_dtypes.bfloat16)
        )
        in_maps.append({
            "xT": np.ascontiguousarray(x[b].T).astype(ml_dtypes.bfloat16),
            "wq": np.ascontiguousarray(Wq[:, cols]).astype(ml_dtypes.bfloat16),
            "wk": np.ascontiguousarray(Wk[:, cols]).astype(ml_dtypes.bfloat16),
            "wv": np.ascontiguousarray(Wv[:, cols]).astype(ml_dtypes.bfloat16),
            "wo2": np.ascontiguousarray(wo2),
            "ident": np.eye(P, dtype=ml_dtypes.bfloat16),
            "ones": np.ones((P, DH), dtype=np.float32),  # f32r param, same bits
            "onesbf": np.ones((P, DH), dtype=ml_dtypes.bfloat16),
        })
    return in_maps


def kernel(x, Wq, Wk, Wv, Wo, bo):
    global LAST_RESULT
    x = np.asarray(x, dtype=np.float32)
    Wq = np.asarray(Wq, dtype=np.float32)
    Wk = np.asarray(Wk, dtype=np.float32)
    Wv = np.asarray(Wv, dtype=np.float32)
    Wo = np.asarray(Wo, dtype=np.float32)
    bo = np.asarray(bo, dtype=np.float32)

    in_maps = make_in_maps(x, Wq, Wk, Wv, Wo)
    nc = _get_nc()
    res = run_bass_kernel_spmd(nc, in_maps, list(range(NCORES)), trace=TRACE)
    LAST_RESULT = res

    out = np.zeros((B, N, D), dtype=np.float32)
    for c in range(NCORES):
        out[c // 4] += res.results[c]["y"]
    out += bo[None, None, :]
    return out
